# revision 33
# baseline (speedup 1.0000x reference)
"""Trainium2 Bass kernel for a 2-layer GRU (H=10) + per-step linear head.

Strategy
--------
Data-parallel: batch B=1024 is sharded 8 ways (128 per core). Within a core
everything is kept "transposed" — hidden units on partitions, batch on the
free dim — so the sequential recurrence never needs a transpose.

Both GRU layers are fused into TWO fp16 matmuls per timestep via combined
block weight matrices acting on a SPLIT state vector
    S = [a ; q ; 1 ; x]      (66 rows incl. alignment pads)
where a = z*h_prev and q = (1-z)*n are kept as separate row blocks with the
h-weights duplicated across both blocks, so the matmul itself computes
W·(a+q) = W·h — the h' = a+q add is OFF the serial critical path (a lazy
consolidation hc = a+q feeds the next tick's z*h product only).
Layer 1 runs one step skewed behind layer 0. The matmuls also produce
negated z pre-activations (so one sigmoid pass yields both z and w=1-z) and
the linear head output y as one extra row.

Per tick the serial critical path is only
  PE:  G1 = Wc1.T @ S(tau-1)          [fp16, single pass]
  ACT: ZRW = sigmoid(G1)              (z|r|w)
  DVE: T0 = ZRW[r] * G2[hn]
  DVE: T1 = T0 + G2[xn]
  ACT: N = tanh(T1)
  DVE: S(tau)[q] = ZRW[w] * N
with the off-path work (a = z*hc, hc = a+q, y copy) scheduled behind it.
The state trajectory lives in a circular SBUF buffer [66, 192, 128]; row 64
is a constant ones row (bias path), row 65 is prefilled with x by DMA.
"""

import numpy as np
from contextlib import ExitStack

import concourse.bass as bass
import concourse.bacc as bacc
import concourse.tile as tile
from concourse import mybir
from concourse.bass_utils import run_bass_kernel_spmd

F32 = mybir.dt.float32
F16 = mybir.dt.float16

H = 10
B = 1024
T = 2048
NCORES = 8
BL = B // NCORES          # 128 batch per core
TC = 192                  # circular trajectory length (multiple of 64)
YB = 64                   # y flush granularity
NTICK = T + 2             # pipeline skew: h0 lags 0, h1 lags 1, y lags 2

# state rows: a (z*h) at 0:20, q ((1-z)*n) at 32:52, ones 64, x 65
SA = 0
SQ = 32
SONE = 64
SX = 65
SROWS = 66


# ----------------------------------------------------------------------------
# host-side weight packing
# ----------------------------------------------------------------------------

def build_wc(Wih0, Whh0, bih0, bhh0, Wih1, Whh1, bih1, bhh1, Wlin, blin):
    """Returns (wc1 [66,116], wc2 [66,65], wc2_init [66,65]) in fp16.

    lhsT rows = split state dims; the h-weights are written identically into
    the a-block (0:20) and q-block (32:52) so the matmul sums them.
    wc2_init zeroes layer-1's n-gate inputs (hn1, xn1) so the tick-0 update
    writes exactly h1 = 0."""
    r, z, n = slice(0, H), slice(H, 2 * H), slice(2 * H, 3 * H)

    def dup(W, cols, hrows, val):
        # write h-weight blocks into both a- and q- row copies
        W[SA + hrows.start:SA + hrows.stop, cols] = val
        W[SQ + hrows.start:SQ + hrows.stop, cols] = val

    h0r, h1r = slice(0, 10), slice(10, 20)

    # G1 columns: z at 0:20, h-passthrough (a+q identity) at 32:52,
    # r at 64:84, w = -z at 96:116.  The h-passthrough rides in G1's PSUM
    # pre-activations (sigma reads them too, writing unused garbage into
    # zrw[32:52]) and feeds the off-path a' = z*h product directly.
    W1 = np.zeros((SROWS, 116), dtype=np.float64)
    # pre_z0 / pre_z1 at cols 0:20
    dup(W1, slice(0, 10), h0r, Whh0[z].T)
    W1[SX, 0:10] = Wih0[z, 0]
    W1[SONE, 0:10] = bih0[z] + bhh0[z]
    dup(W1, slice(10, 20), h1r, Whh1[z].T)
    dup(W1, slice(10, 20), h0r, Wih1[z].T)
    W1[SONE, 10:20] = bih1[z] + bhh1[z]
    # h-passthrough identity at cols 32:52
    for i in range(20):
        W1[SA + i, 32 + i] = 1.0
        W1[SQ + i, 32 + i] = 1.0
    # pre_r0 / pre_r1 at cols 64:84
    dup(W1, slice(64, 74), h0r, Whh0[r].T)
    W1[SX, 64:74] = Wih0[r, 0]
    W1[SONE, 64:74] = bih0[r] + bhh0[r]
    dup(W1, slice(74, 84), h1r, Whh1[r].T)
    dup(W1, slice(74, 84), h0r, Wih1[r].T)
    W1[SONE, 74:84] = bih1[r] + bhh1[r]
    # negated z blocks at 96:116 (w = sigmoid(-pre_z) = 1 - z)
    W1[:, 96:116] = -W1[:, 0:20]

    # G2 columns: hn 0:20, xn 32:52, y 64
    W2 = np.zeros((SROWS, 65), dtype=np.float64)
    # hn0 / hn1 at cols 0:20
    dup(W2, slice(0, 10), h0r, Whh0[n].T)
    W2[SONE, 0:10] = bhh0[n]
    dup(W2, slice(10, 20), h1r, Whh1[n].T)
    W2[SONE, 10:20] = bhh1[n]
    # xn0 / xn1 at cols 32:52
    W2[SX, 32:42] = Wih0[n, 0]
    W2[SONE, 32:42] = bih0[n]
    dup(W2, slice(42, 52), h0r, Wih1[n].T)
    W2[SONE, 42:52] = bih1[n]
    # y row at col 64
    dup(W2, slice(64, 65), h1r, Wlin[0][:, None])
    W2[SONE, 64] = blin[0]

    W2i = W2.copy()
    W2i[:, 10:20] = 0.0
    W2i[:, 42:52] = 0.0

    return (np.ascontiguousarray(W1, dtype=np.float16),
            np.ascontiguousarray(W2, dtype=np.float16),
            np.ascontiguousarray(W2i, dtype=np.float16))


# ----------------------------------------------------------------------------
# device program
# ----------------------------------------------------------------------------

def _emit_x_prefill(nc, xt_ap, straj, m):
    """DMA filling x-row group m: trajectory row SX, cols
    [64*(m%3), 64*(m%3)+64) <- x(64m+1 .. 64m+64). Clipped to x rows < T+2."""
    j0 = 64 * (m % (TC // 64))
    s0 = 64 * m + 1
    ncols = min(64, (T + 2) - s0)
    if ncols <= 0:
        return
    nc.sync.dma_start(
        out=straj[SX:SX + 1, j0:j0 + ncols, :],
        in_=xt_ap[s0:s0 + ncols, :],
    )


def build_program(n_tick=NTICK):
    nc = bacc.Bacc("TRN2", target_bir_lowering=False, debug=False)
    xt_ap = nc.dram_tensor("xT", [T + 2, BL], F16, kind="ExternalInput").ap()
    wc1_ap = nc.dram_tensor("wc1", [SROWS, 116], F16, kind="ExternalInput").ap()
    wc2_ap = nc.dram_tensor("wc2", [SROWS, 65], F16, kind="ExternalInput").ap()
    wc2i_ap = nc.dram_tensor("wc2i", [SROWS, 65], F16,
                             kind="ExternalInput").ap()
    ones_ap = nc.dram_tensor("ones_row", [1, TC * BL], F16,
                             kind="ExternalInput").ap()
    zpad_ap = nc.dram_tensor("zeros_pad", [12, TC * BL], F16,
                             kind="ExternalInput").ap()
    init_ap = nc.dram_tensor("init_col", [SROWS, BL], F16,
                             kind="ExternalInput").ap()
    yt_ap = nc.dram_tensor("yT", [T, BL], F32, kind="ExternalOutput").ap()

    with tile.TileContext(nc) as tc, ExitStack() as ctx:
        consts = ctx.enter_context(tc.tile_pool(name="consts", bufs=1))
        traj = ctx.enter_context(tc.tile_pool(name="traj", bufs=1))
        psum = ctx.enter_context(tc.tile_pool(name="psum", bufs=2, space="PSUM"))
        psum2 = ctx.enter_context(tc.tile_pool(name="psum2", bufs=2,
                                               space="PSUM"))
        work = ctx.enter_context(tc.tile_pool(name="work", bufs=3))
        ypool = ctx.enter_context(tc.tile_pool(name="ybuf", bufs=2))

        wc1_sb = consts.tile([SROWS, 116], F16)
        nc.scalar.dma_start(out=wc1_sb[:, :], in_=wc1_ap)
        wc2_sb = consts.tile([SROWS, 65], F16)
        nc.gpsimd.dma_start(out=wc2_sb[:, :], in_=wc2_ap)
        wc2i_sb = consts.tile([SROWS, 65], F16)
        nc.sync.dma_start(out=wc2i_sb[:, :], in_=wc2i_ap)

        straj = traj.tile([SROWS, TC, BL], F16)
        # zero the alignment-pad rows via DMA (a ~26us DVE memset would gate
        # the first tick); they feed the matmul with zero weights and must
        # not contain NaN garbage. The a/q rows need no zeroing: every
        # column is written by its tick before any matmul reads it (col
        # TC-1 comes from the init DMA).
        _qs = (nc.sync, nc.scalar, nc.gpsimd)
        for j, base in enumerate((20, 52)):
            for k in range(4):
                c0 = 48 * k
                _qs[(4 * j + k) % 3].dma_start(
                    out=straj[base:base + 12, c0:c0 + 48, :],
                    in_=zpad_ap[:, c0 * BL:(c0 + 48) * BL])
        # initial state column (a=q=0, one, x(0)) read by tick 0; ones row
        # everywhere else (bias path of the combined matmul)
        nc.scalar.dma_start(out=straj[0:SROWS, TC - 1, :], in_=init_ap)
        for q in range(TC // 64):
            _qs[q % 3].dma_start(out=straj[SONE:SONE + 1, 64 * q:64 * (q + 1), :],
                                 in_=ones_ap[0:1, 64 * q * BL:64 * (q + 1) * BL])
        # groups 0..1 cover x(1..128) for ticks 1..128
        for m in range(2):
            _emit_x_prefill(nc, xt_ap, straj, m)
        # group 2 minus its last col (col TC-1 still holds x(0) until tick 0)
        nc.gpsimd.dma_start(out=straj[SX:SX + 1, 128:TC - 1, :],
                            in_=xt_ap[129:TC, :])

        ybuf = None
        sig = mybir.ActivationFunctionType.Sigmoid
        tnh = mybir.ActivationFunctionType.Tanh

        for tau in range(n_tick):
            col_r = (tau - 1) % TC
            col_w = tau % TC

            g1 = psum.tile([116, BL], F32)
            nc.tensor.matmul(
                g1[:, :], wc1_sb[:, :], straj[0:SROWS, col_r, :],
                start=True, stop=True,
            )
            g2 = psum.tile([65, BL], F32)
            nc.tensor.matmul(
                g2[:, :],
                wc2i_sb[:, :] if tau == 0 else wc2_sb[:, :],
                straj[0:SROWS, col_r, :],
                start=True, stop=True,
            )

            # zrw rows: z at 0:20, sigma(h) garbage at 32:52, r at 64:84,
            # w=1-z at 96:116
            zrw = work.tile([116, BL], F16)
            nc.scalar.activation(zrw[:, :], g1[:, :], sig)

            t0 = work.tile([20, BL], F32)
            nc.vector.tensor_mul(t0[:, :], zrw[64:84, :], g2[0:20, :])
            # t1 lands in PSUM so tanh takes the scalar engine's cheaper
            # PSUM-read path (172 vs 222 access cycles)
            t1 = psum2.tile([20, BL], F32)
            nc.vector.tensor_add(t1[:, :], t0[:, :], g2[32:52, :])
            # evacuate consolidated h (G1's identity passthrough) to SBUF on
            # the scalar engine, which idles between sigmoid and tanh; this
            # keeps the a' product an all-SBUF fp16 op
            hsb = work.tile([20, BL], F16)
            nc.scalar.copy(hsb[:, :], g1[32:52, :])
            nt = work.tile([116, BL], F16)
            nc.scalar.activation(nt[96:116, :], t1[:, :], tnh)

            # off-path (runs on DVE while tanh runs on ACT):
            # a(tau) = z * h(tau-1)
            nc.vector.tensor_mul(straj[SA:SA + 20, col_w, :],
                                 zrw[0:20, :], hsb[:, :])
            # q(tau) = w * n  — the last op on the serial critical path
            nc.vector.tensor_mul(straj[SQ:SQ + 20, col_w, :],
                                 zrw[96:116, :], nt[96:116, :])

            # y(tau-2) emerges as G2 row 64
            s = tau - 2
            if 0 <= s < T:
                if s % YB == 0:
                    ybuf = ypool.tile([1, YB * BL], F32)
                nc.scalar.copy(
                    ybuf[0:1, (s % YB) * BL:(s % YB) * BL + BL],
                    g2[64:65, :],
                )
                if s % YB == YB - 1:
                    nc.sync.dma_start(
                        out=yt_ap[s - (YB - 1):s + 1, :],
                        in_=ybuf[0:1, :].rearrange("p (t b) -> p t b", b=BL),
                    )

            if tau == 0:
                # col TC-1's x slot is free now: x(TC) for tick TC
                nc.sync.dma_start(out=straj[SX:SX + 1, TC - 1:TC, :],
                                  in_=xt_ap[TC:TC + 1, :])
            # steady-state prefill: at tick 64j+1 (j>=1) issue group j+2
            if tau >= 65 and tau % 64 == 1:
                _emit_x_prefill(nc, xt_ap, straj, (tau - 1) // 64 + 2)

    nc.compile()
    return nc


_program_cache = {}


def _get_program(n_tick=NTICK):
    if n_tick not in _program_cache:
        _program_cache[n_tick] = build_program(n_tick)
    return _program_cache[n_tick]


def make_in_maps(x, weights):
    wc1, wc2, wc2i = build_wc(*weights)
    x = np.asarray(x, dtype=np.float32)
    xt = np.zeros((T + 2, B), dtype=np.float16)
    xt[:T] = x[:, :, 0].T  # [T, B]
    ones_row = np.ones((1, TC * BL), dtype=np.float16)
    zeros_pad = np.zeros((12, TC * BL), dtype=np.float16)
    in_maps = []
    for c in range(NCORES):
        xc = np.ascontiguousarray(xt[:, c * BL:(c + 1) * BL])
        init_col = np.zeros((SROWS, BL), dtype=np.float16)
        init_col[SONE] = 1.0
        init_col[SX] = xc[0]
        in_maps.append({"xT": xc, "wc1": wc1, "wc2": wc2, "wc2i": wc2i,
                        "ones_row": ones_row, "zeros_pad": zeros_pad,
                        "init_col": init_col})
    return in_maps


# ----------------------------------------------------------------------------
# host entry point
# ----------------------------------------------------------------------------

def kernel(x, Wih0, Whh0, bih0, bhh0, Wih1, Whh1, bih1, bhh1, Wlin, blin,
           _trace=False, _trace_kwargs=None):
    weights = [np.asarray(a) for a in
               (Wih0, Whh0, bih0, bhh0, Wih1, Whh1, bih1, bhh1, Wlin, blin)]
    in_maps = make_in_maps(x, weights)
    nc = _get_program()
    res = run_bass_kernel_spmd(
        nc, in_maps, core_ids=list(range(NCORES)),
        trace=_trace, **(_trace_kwargs or {}),
    )
    results = res.results if hasattr(res, "results") else res
    yt = np.concatenate([results[c]["yT"] for c in range(NCORES)], axis=1)
    out = np.ascontiguousarray(yt.T)[:, :, None].astype(np.float32)
    if _trace:
        return out, res
    return out


# revision 34
# speedup vs baseline: 1.0008x; 1.0008x over previous
"""Trainium2 Bass kernel for a 2-layer GRU (H=10) + per-step linear head.

Strategy
--------
Data-parallel: batch B=1024 is sharded 8 ways (128 per core). Within a core
everything is kept "transposed" — hidden units on partitions, batch on the
free dim — so the sequential recurrence never needs a transpose.

Both GRU layers are fused into TWO fp16 matmuls per timestep via combined
block weight matrices acting on a SPLIT state vector
    S = [a ; q ; 1 ; x]      (66 rows incl. alignment pads)
where a = z*h_prev and q = (1-z)*n are kept as separate row blocks with the
h-weights duplicated across both blocks, so the matmul itself computes
W·(a+q) = W·h — the h' = a+q add is OFF the serial critical path (a lazy
consolidation hc = a+q feeds the next tick's z*h product only).
Layer 1 runs one step skewed behind layer 0. The matmuls also produce
negated z pre-activations (so one sigmoid pass yields both z and w=1-z) and
the linear head output y as one extra row.

Per tick the serial critical path is only
  PE:  G1 = Wc1.T @ S(tau-1)          [fp16, single pass]
  ACT: ZRW = sigmoid(G1)              (z|r|w)
  DVE: T0 = ZRW[r] * G2[hn]
  DVE: T1 = T0 + G2[xn]
  ACT: N = tanh(T1)
  DVE: S(tau)[q] = ZRW[w] * N
with the off-path work (a = z*hc, hc = a+q, y copy) scheduled behind it.
The state trajectory lives in a circular SBUF buffer [66, 192, 128]; row 64
is a constant ones row (bias path), row 65 is prefilled with x by DMA.
"""

import numpy as np
from contextlib import ExitStack

import concourse.bass as bass
import concourse.bacc as bacc
import concourse.tile as tile
from concourse import mybir
from concourse.bass_utils import run_bass_kernel_spmd

F32 = mybir.dt.float32
F16 = mybir.dt.float16

H = 10
B = 1024
T = 2048
NCORES = 8
BL = B // NCORES          # 128 batch per core
TC = 192                  # circular trajectory length (multiple of 64)
YB = 64                   # y flush granularity
NTICK = T + 2             # pipeline skew: h0 lags 0, h1 lags 1, y lags 2

# state rows: a (z*h) at 0:20, q ((1-z)*n) at 32:52, ones 64, x 65
SA = 0
SQ = 32
SONE = 64
SX = 65
SROWS = 66


# ----------------------------------------------------------------------------
# host-side weight packing
# ----------------------------------------------------------------------------

def build_wc(Wih0, Whh0, bih0, bhh0, Wih1, Whh1, bih1, bhh1, Wlin, blin):
    """Returns (wc1 [66,116], wc2 [66,65], wc2_init [66,65]) in fp16.

    lhsT rows = split state dims; the h-weights are written identically into
    the a-block (0:20) and q-block (32:52) so the matmul sums them.
    wc2_init zeroes layer-1's n-gate inputs (hn1, xn1) so the tick-0 update
    writes exactly h1 = 0."""
    r, z, n = slice(0, H), slice(H, 2 * H), slice(2 * H, 3 * H)

    def dup(W, cols, hrows, val):
        # write h-weight blocks into both a- and q- row copies
        W[SA + hrows.start:SA + hrows.stop, cols] = val
        W[SQ + hrows.start:SQ + hrows.stop, cols] = val

    h0r, h1r = slice(0, 10), slice(10, 20)

    # G1 columns: z at 0:20, h-passthrough (a+q identity) at 32:52,
    # r at 64:84, w = -z at 96:116.  The h-passthrough rides in G1's PSUM
    # pre-activations (sigma reads them too, writing unused garbage into
    # zrw[32:52]) and feeds the off-path a' = z*h product directly.
    W1 = np.zeros((SROWS, 116), dtype=np.float64)
    # pre_z0 / pre_z1 at cols 0:20
    dup(W1, slice(0, 10), h0r, Whh0[z].T)
    W1[SX, 0:10] = Wih0[z, 0]
    W1[SONE, 0:10] = bih0[z] + bhh0[z]
    dup(W1, slice(10, 20), h1r, Whh1[z].T)
    dup(W1, slice(10, 20), h0r, Wih1[z].T)
    W1[SONE, 10:20] = bih1[z] + bhh1[z]
    # h-passthrough identity at cols 32:52
    for i in range(20):
        W1[SA + i, 32 + i] = 1.0
        W1[SQ + i, 32 + i] = 1.0
    # pre_r0 / pre_r1 at cols 64:84
    dup(W1, slice(64, 74), h0r, Whh0[r].T)
    W1[SX, 64:74] = Wih0[r, 0]
    W1[SONE, 64:74] = bih0[r] + bhh0[r]
    dup(W1, slice(74, 84), h1r, Whh1[r].T)
    dup(W1, slice(74, 84), h0r, Wih1[r].T)
    W1[SONE, 74:84] = bih1[r] + bhh1[r]
    # negated z blocks at 96:116 (w = sigmoid(-pre_z) = 1 - z)
    W1[:, 96:116] = -W1[:, 0:20]

    # G2 columns: hn 0:20, xn 32:52, y 64
    W2 = np.zeros((SROWS, 65), dtype=np.float64)
    # hn0 / hn1 at cols 0:20
    dup(W2, slice(0, 10), h0r, Whh0[n].T)
    W2[SONE, 0:10] = bhh0[n]
    dup(W2, slice(10, 20), h1r, Whh1[n].T)
    W2[SONE, 10:20] = bhh1[n]
    # xn0 / xn1 at cols 32:52
    W2[SX, 32:42] = Wih0[n, 0]
    W2[SONE, 32:42] = bih0[n]
    dup(W2, slice(42, 52), h0r, Wih1[n].T)
    W2[SONE, 42:52] = bih1[n]
    # y row at col 64
    dup(W2, slice(64, 65), h1r, Wlin[0][:, None])
    W2[SONE, 64] = blin[0]

    W2i = W2.copy()
    W2i[:, 10:20] = 0.0
    W2i[:, 42:52] = 0.0

    return (np.ascontiguousarray(W1, dtype=np.float16),
            np.ascontiguousarray(W2, dtype=np.float16),
            np.ascontiguousarray(W2i, dtype=np.float16))


# ----------------------------------------------------------------------------
# device program
# ----------------------------------------------------------------------------

def _emit_x_prefill(nc, xt_ap, straj, m):
    """DMA filling x-row group m: trajectory row SX, cols
    [64*(m%3), 64*(m%3)+64) <- x(64m+1 .. 64m+64). Clipped to x rows < T+2."""
    j0 = 64 * (m % (TC // 64))
    s0 = 64 * m + 1
    ncols = min(64, (T + 2) - s0)
    if ncols <= 0:
        return
    nc.sync.dma_start(
        out=straj[SX:SX + 1, j0:j0 + ncols, :],
        in_=xt_ap[s0:s0 + ncols, :],
    )


def build_program(n_tick=NTICK):
    nc = bacc.Bacc("TRN2", target_bir_lowering=False, debug=False)
    xt_ap = nc.dram_tensor("xT", [T + 2, BL], F16, kind="ExternalInput").ap()
    wc1_ap = nc.dram_tensor("wc1", [SROWS, 116], F16, kind="ExternalInput").ap()
    wc2_ap = nc.dram_tensor("wc2", [SROWS, 65], F16, kind="ExternalInput").ap()
    wc2i_ap = nc.dram_tensor("wc2i", [SROWS, 65], F16,
                             kind="ExternalInput").ap()
    ones_ap = nc.dram_tensor("ones_row", [1, TC * BL], F16,
                             kind="ExternalInput").ap()
    zpad_ap = nc.dram_tensor("zeros_pad", [12, TC * BL], F16,
                             kind="ExternalInput").ap()
    init_ap = nc.dram_tensor("init_col", [SROWS, BL], F16,
                             kind="ExternalInput").ap()
    yt_ap = nc.dram_tensor("yT", [T, BL], F32, kind="ExternalOutput").ap()

    with tile.TileContext(nc) as tc, ExitStack() as ctx:
        consts = ctx.enter_context(tc.tile_pool(name="consts", bufs=1))
        traj = ctx.enter_context(tc.tile_pool(name="traj", bufs=1))
        psum = ctx.enter_context(tc.tile_pool(name="psum", bufs=2, space="PSUM"))
        psum2 = ctx.enter_context(tc.tile_pool(name="psum2", bufs=2,
                                               space="PSUM"))
        work = ctx.enter_context(tc.tile_pool(name="work", bufs=3))
        ypool = ctx.enter_context(tc.tile_pool(name="ybuf", bufs=2))

        wc1_sb = consts.tile([SROWS, 116], F16)
        nc.sync.dma_start(out=wc1_sb[:, :], in_=wc1_ap)
        wc2_sb = consts.tile([SROWS, 65], F16)
        nc.sync.dma_start(out=wc2_sb[:, :], in_=wc2_ap)
        wc2i_sb = consts.tile([SROWS, 65], F16)
        nc.sync.dma_start(out=wc2i_sb[:, :], in_=wc2i_ap)

        straj = traj.tile([SROWS, TC, BL], F16)
        # zero the alignment-pad rows via DMA (a ~26us DVE memset would gate
        # the first tick); they feed the matmul with zero weights and must
        # not contain NaN garbage. The a/q rows need no zeroing: every
        # column is written by its tick before any matmul reads it (col
        # TC-1 comes from the init DMA).
        _qs = (nc.sync, nc.scalar, nc.gpsimd)
        for j, base in enumerate((20, 52)):
            for k in range(4):
                c0 = 48 * k
                _qs[(4 * j + k) % 3].dma_start(
                    out=straj[base:base + 12, c0:c0 + 48, :],
                    in_=zpad_ap[:, c0 * BL:(c0 + 48) * BL])
        # initial state column (a=q=0, one, x(0)) read by tick 0; ones row
        # everywhere else (bias path of the combined matmul)
        nc.sync.dma_start(out=straj[0:SROWS, TC - 1, :], in_=init_ap)
        for q in range(TC // 64):
            nc.sync.dma_start(out=straj[SONE:SONE + 1, 64 * q:64 * (q + 1), :],
                              in_=ones_ap[0:1, 64 * q * BL:64 * (q + 1) * BL])
        # groups 0..1 cover x(1..128) for ticks 1..128
        for m in range(2):
            _emit_x_prefill(nc, xt_ap, straj, m)
        # group 2 minus its last col (col TC-1 still holds x(0) until tick 0)
        nc.sync.dma_start(out=straj[SX:SX + 1, 128:TC - 1, :],
                          in_=xt_ap[129:TC, :])

        ybuf = None
        sig = mybir.ActivationFunctionType.Sigmoid
        tnh = mybir.ActivationFunctionType.Tanh

        for tau in range(n_tick):
            col_r = (tau - 1) % TC
            col_w = tau % TC

            g1 = psum.tile([116, BL], F32)
            nc.tensor.matmul(
                g1[:, :], wc1_sb[:, :], straj[0:SROWS, col_r, :],
                start=True, stop=True,
            )
            g2 = psum.tile([65, BL], F32)
            nc.tensor.matmul(
                g2[:, :],
                wc2i_sb[:, :] if tau == 0 else wc2_sb[:, :],
                straj[0:SROWS, col_r, :],
                start=True, stop=True,
            )

            # zrw rows: z at 0:20, sigma(h) garbage at 32:52, r at 64:84,
            # w=1-z at 96:116
            zrw = work.tile([116, BL], F16)
            nc.scalar.activation(zrw[:, :], g1[:, :], sig)

            t0 = work.tile([20, BL], F32)
            nc.vector.tensor_mul(t0[:, :], zrw[64:84, :], g2[0:20, :])
            # t1 lands in PSUM so tanh takes the scalar engine's cheaper
            # PSUM-read path (172 vs 222 access cycles)
            t1 = psum2.tile([20, BL], F32)
            nc.vector.tensor_add(t1[:, :], t0[:, :], g2[32:52, :])
            # evacuate consolidated h (G1's identity passthrough) to SBUF on
            # the scalar engine, which idles between sigmoid and tanh; this
            # keeps the a' product an all-SBUF fp16 op
            hsb = work.tile([20, BL], F16)
            nc.scalar.copy(hsb[:, :], g1[32:52, :])
            nt = work.tile([116, BL], F16)
            nc.scalar.activation(nt[96:116, :], t1[:, :], tnh)

            # off-path (runs on DVE while tanh runs on ACT):
            # a(tau) = z * h(tau-1)
            nc.vector.tensor_mul(straj[SA:SA + 20, col_w, :],
                                 zrw[0:20, :], hsb[:, :])
            # q(tau) = w * n  — the last op on the serial critical path
            nc.vector.tensor_mul(straj[SQ:SQ + 20, col_w, :],
                                 zrw[96:116, :], nt[96:116, :])

            # y(tau-2) emerges as G2 row 64
            s = tau - 2
            if 0 <= s < T:
                if s % YB == 0:
                    ybuf = ypool.tile([1, YB * BL], F32)
                nc.scalar.copy(
                    ybuf[0:1, (s % YB) * BL:(s % YB) * BL + BL],
                    g2[64:65, :],
                )
                if s % YB == YB - 1:
                    nc.sync.dma_start(
                        out=yt_ap[s - (YB - 1):s + 1, :],
                        in_=ybuf[0:1, :].rearrange("p (t b) -> p t b", b=BL),
                    )

            if tau == 0:
                # col TC-1's x slot is free now: x(TC) for tick TC
                nc.sync.dma_start(out=straj[SX:SX + 1, TC - 1:TC, :],
                                  in_=xt_ap[TC:TC + 1, :])
            # steady-state prefill: at tick 64j+1 (j>=1) issue group j+2
            if tau >= 65 and tau % 64 == 1:
                _emit_x_prefill(nc, xt_ap, straj, (tau - 1) // 64 + 2)

    nc.compile()
    return nc


_program_cache = {}


def _get_program(n_tick=NTICK):
    if n_tick not in _program_cache:
        _program_cache[n_tick] = build_program(n_tick)
    return _program_cache[n_tick]


def make_in_maps(x, weights):
    wc1, wc2, wc2i = build_wc(*weights)
    x = np.asarray(x, dtype=np.float32)
    xt = np.zeros((T + 2, B), dtype=np.float16)
    xt[:T] = x[:, :, 0].T  # [T, B]
    ones_row = np.ones((1, TC * BL), dtype=np.float16)
    zeros_pad = np.zeros((12, TC * BL), dtype=np.float16)
    in_maps = []
    for c in range(NCORES):
        xc = np.ascontiguousarray(xt[:, c * BL:(c + 1) * BL])
        init_col = np.zeros((SROWS, BL), dtype=np.float16)
        init_col[SONE] = 1.0
        init_col[SX] = xc[0]
        in_maps.append({"xT": xc, "wc1": wc1, "wc2": wc2, "wc2i": wc2i,
                        "ones_row": ones_row, "zeros_pad": zeros_pad,
                        "init_col": init_col})
    return in_maps


# ----------------------------------------------------------------------------
# host entry point
# ----------------------------------------------------------------------------

def kernel(x, Wih0, Whh0, bih0, bhh0, Wih1, Whh1, bih1, bhh1, Wlin, blin,
           _trace=False, _trace_kwargs=None):
    weights = [np.asarray(a) for a in
               (Wih0, Whh0, bih0, bhh0, Wih1, Whh1, bih1, bhh1, Wlin, blin)]
    in_maps = make_in_maps(x, weights)
    nc = _get_program()
    res = run_bass_kernel_spmd(
        nc, in_maps, core_ids=list(range(NCORES)),
        trace=_trace, **(_trace_kwargs or {}),
    )
    results = res.results if hasattr(res, "results") else res
    yt = np.concatenate([results[c]["yT"] for c in range(NCORES)], axis=1)
    out = np.ascontiguousarray(yt.T)[:, :, None].astype(np.float32)
    if _trace:
        return out, res
    return out
